# revision 5
# baseline (speedup 1.0000x reference)
"""Trainium2 Bass kernel for nn_NegF1: distributed -F1 loss over 16.7M elements.

Data-parallel over 8 NeuronCores; each core streams its 2,097,152-element
slice of probs (f32) / lbls (int32) from HBM (probs on the sync HWDGE ring,
lbls on the scalar ring). Memory-bound: every compute engine is kept under
~60% of the DMA budget so the input stream never stalls.

Per [128, F] tile, DVE writes three bf16 planes into one interleaved
"comb" buffer laid out [128][chunk c][slot s][128] (chunk = 128 columns):
  slot 0: pb = bf16(p)                 (tensor_copy, full rate)
  slot 1: y  = (p > .5) * pb           (scalar_tensor_tensor with fused
                                        accum -> Y = sum(g*p); half-rate
                                        but replaces plane + ones-matmul)
  slot 2: g  = is_gt(p, 0.5)           (computed on fp32 p -> bit-exact)
ACT casts lb = bf16(l) with fused accum -> Npos.

The otherwise-idle TensorEngine does the l-weighted reductions via the
diag trick: lhsT = lb chunk [128,128], rhs = comb chunk [128, 3*128],
accumulated into PSUM [128, 384] (two banks, alternating chunks); diag of
block s over chunks gives Sx = sum(l*pb), TP = sum(l*y), C = sum(l*g).
No ones-matmuls: Y comes from the DVE fused accum. A burst of zero
matmuls at the start warms the PE HAM clock-gate during the first DMAs.

First/last tiles are tapered to shorten pipeline fill and drain.

Host combine (float64):
  FP = Y - TP;  FN = Npos - Sx - C + TP
  f1 from TP/FP/FN with eps=1e-5;  return -f1 as float32 scalar.
"""

from contextlib import ExitStack

import numpy as np

N_TOTAL = 16777216
N_CORES = 8
M_PER_CORE = N_TOTAL // N_CORES   # 2097152
P = 128                           # SBUF partitions
EPS = 1e-05
CH = 128                          # diag chunk columns

_CACHE = {}


def build_nc(M=M_PER_CORE, F=2048, bufs=3, in_bufs=6, warmup_mms=12,
             debug=False):
    import concourse.bacc as bacc
    import concourse.mybir as mybir
    import concourse.tile as tile

    cols = M // P                 # 16384
    start_taper = [F // 4, F // 4]
    end_taper = [F // 4, F // 8, F // 8]
    body = (cols - sum(start_taper) - sum(end_taper)) // F
    tiles = start_taper + [F] * body + end_taper
    assert sum(tiles) == cols and all(Ft % CH == 0 for Ft in tiles)
    T = len(tiles)

    f32 = mybir.dt.float32
    i32 = mybir.dt.int32
    bf16 = mybir.dt.bfloat16
    Alu = mybir.AluOpType
    Act = mybir.ActivationFunctionType

    nc = bacc.Bacc("TRN2", target_bir_lowering=False, debug=debug,
                   num_devices=N_CORES)

    probs = nc.dram_tensor("probs", [M], f32, kind="ExternalInput")
    lbls = nc.dram_tensor("lbls", [M], i32, kind="ExternalInput")
    out_diag = nc.dram_tensor("out_diag", [P, 2 * 3 * CH], bf16,
                              kind="ExternalOutput")
    out_accN = nc.dram_tensor("out_accN", [P, T], f32, kind="ExternalOutput")
    out_accY = nc.dram_tensor("out_accY", [P, T], f32, kind="ExternalOutput")

    # per-tile DRAM views: tile t is one contiguous block of P*Ft elements
    def tile_view(ap_flat, start_el, Ft):
        return ap_flat[start_el:start_el + P * Ft].rearrange(
            "(p f) -> p f", p=P, f=Ft)

    p1 = probs.ap()
    l1 = lbls.ap()

    with tile.TileContext(nc) as tc, ExitStack() as ctx:
        pin = ctx.enter_context(tc.tile_pool(name="pin", bufs=in_bufs))
        lin = ctx.enter_context(tc.tile_pool(name="lin", bufs=in_bufs))
        lbpool = ctx.enter_context(tc.tile_pool(name="lbpool", bufs=bufs))
        cpool = ctx.enter_context(tc.tile_pool(name="cpool", bufs=bufs))
        accp = ctx.enter_context(tc.tile_pool(name="accp", bufs=1))
        psump = ctx.enter_context(tc.tile_pool(name="psump", bufs=1,
                                               space="PSUM"))

        accN = accp.tile([P, T], f32)
        accY = accp.tile([P, T], f32)

        # two alternating diag accumulators (even / odd chunks) so
        # back-to-back accumulating matmuls don't chain on one PSUM bank
        ps_diag0 = psump.tile([P, 3 * CH], f32)
        ps_diag1 = psump.tile([P, 3 * CH], f32)

        # Warm the PE HAM clock-gate (1.2 -> 2.4 GHz needs ~3.4us sustained)
        # while the first input DMAs are in flight.
        if warmup_mms:
            wu = accp.tile([P, 3 * CH], bf16)
            nc.vector.memset(wu[:], 0.0)
            ps_wu = psump.tile([P, 3 * CH], f32)
            for i in range(warmup_mms):
                nc.tensor.matmul(ps_wu[:, :], wu[:, :CH], wu[:],
                                 start=(i == 0), stop=(i == warmup_mms - 1))

        nctot = cols // CH              # total diag chunks (128)
        bank = [i % 2 for i in range(nctot)]
        b0_stop = max(i for i, b in enumerate(bank) if b == 0)
        b1_stop = max(i for i, b in enumerate(bank) if b == 1)
        ci = 0
        off = 0
        diag_sb = accp.tile([P, 2 * 3 * CH], bf16)
        for t, Ft in enumerate(tiles):
            NCt = Ft // CH
            start_el = P * off
            off += Ft

            # Taper tiles get their own slot sets so their DMAs queue
            # immediately at the start and prefetch early at the end.
            tap = "_tap" if Ft < F else ""
            pt = pin.tile([P, Ft], f32, tag="pt" + tap, name=f"pt{t}")
            nc.sync.dma_start(out=pt[:, :Ft], in_=tile_view(p1, start_el, Ft))
            lt = lin.tile([P, Ft], i32, tag="lt" + tap, name=f"lt{t}")
            nc.scalar.dma_start(out=lt[:, :Ft], in_=tile_view(l1, start_el, Ft))

            # lb = bf16(l); fused accum -> Npos partials
            lb = lbpool.tile([P, F], bf16, tag="lb")
            nc.scalar.activation(lb[:, :Ft], lt[:, :Ft], Act.Copy,
                                 accum_out=accN[:, t:t + 1])

            comb = cpool.tile([P, 3 * F], bf16, tag="comb")
            c4 = comb[:].rearrange("p (c s j) -> p c s j", c=F // CH, s=3,
                                   j=CH)[:, :NCt]
            pt4 = pt[:, :Ft].rearrange("p (c j) -> p c j", c=NCt, j=CH)

            # slot 0: pb = bf16(p)
            nc.vector.tensor_copy(out=c4[:, :, 0, :], in_=pt4)
            # slot 2: g = [p > 0.5]  (fp32 compare)
            nc.vector.tensor_scalar(out=c4[:, :, 2, :], in0=pt4,
                                    scalar1=0.5, scalar2=None, op0=Alu.is_gt)
            # slot 1: y = g * pb with fused accum -> Y partials
            nc.vector.scalar_tensor_tensor(
                out=c4[:, :, 1, :], in0=pt4, scalar=0.5, in1=c4[:, :, 0, :],
                op0=Alu.is_gt, op1=Alu.mult, accum_out=accY[:, t:t + 1])

            # diag reductions: ps_diag{0,1} += lb_c.T @ comb_c
            for c in range(NCt):
                ps = ps_diag0 if bank[ci] == 0 else ps_diag1
                nc.tensor.matmul(
                    ps[:, :], lb[:, c * CH:(c + 1) * CH],
                    c4[:, c, :, :],
                    start=(ci in (0, 1)),
                    stop=(ci in (b0_stop, b1_stop)))
                ci += 1

        # PSUM -> SBUF (bf16, halves the output DMA) -> DRAM
        nc.scalar.activation(diag_sb[:, :3 * CH], ps_diag0[:, :], Act.Copy)
        nc.vector.tensor_copy(out=diag_sb[:, 3 * CH:], in_=ps_diag1[:, :])

        nc.sync.dma_start(out=out_diag.ap(), in_=diag_sb[:])
        nc.sync.dma_start(out=out_accY.ap(), in_=accY[:])
        nc.scalar.dma_start(out=out_accN.ap(), in_=accN[:])

    nc.compile()
    return nc, T


def get_nc():
    if "nc" not in _CACHE:
        _CACHE["nc"] = build_nc()
    return _CACHE["nc"]


def run_device(probs, lbls, trace=False, **run_kwargs):
    """Run the SPMD kernel; returns (per-core result dicts, BassKernelResults)."""
    from concourse import bass_utils

    nc, _ = get_nc()
    probs = np.ascontiguousarray(probs, dtype=np.float32)
    lbls = np.ascontiguousarray(lbls, dtype=np.int32)
    assert probs.shape == (N_TOTAL,) and lbls.shape == (N_TOTAL,)
    M = M_PER_CORE
    in_maps = [
        {"probs": probs[c * M:(c + 1) * M], "lbls": lbls[c * M:(c + 1) * M]}
        for c in range(N_CORES)
    ]
    res = bass_utils.run_bass_kernel_spmd(
        nc, in_maps, core_ids=list(range(N_CORES)), trace=trace, **run_kwargs)
    return res.results, res


def combine(results):
    """Combine per-core partial sums into the final -f1 scalar."""
    Npos = Y = Sx = TP = C = 0.0
    for r in results:
        dg = np.asarray(r["out_diag"], dtype=np.float64).reshape(P, 2, 3, CH)
        for b in range(2):
            Sx += np.trace(dg[:, b, 0, :])
            TP += np.trace(dg[:, b, 1, :])
            C += np.trace(dg[:, b, 2, :])
        Npos += np.asarray(r["out_accN"], dtype=np.float64).sum()
        Y += np.asarray(r["out_accY"], dtype=np.float64).sum()

    FP = Y - TP
    FN = Npos - Sx - C + TP
    precision = (TP + EPS) / (TP + FP + EPS)
    recall = (TP + EPS) / (TP + FN + EPS)
    f1 = 2.0 * precision * recall / (precision + recall)
    return np.float32(-f1)


def kernel(probs, lbls):
    results, _ = run_device(probs, lbls)
    return np.asarray(combine(results), dtype=np.float32)


if __name__ == "__main__":
    rng = np.random.default_rng(0)
    p = rng.uniform(0, 1, N_TOTAL).astype(np.float32)
    l = rng.integers(0, 2, N_TOTAL).astype(np.int32)
    out = kernel(p, l)
    print("kernel output:", out)


# revision 6
# speedup vs baseline: 1.0136x; 1.0136x over previous
"""Trainium2 Bass kernel for nn_NegF1: distributed -F1 loss over 16.7M elements.

Data-parallel over 8 NeuronCores; each core streams its 2,097,152-element
slice of probs (f32) / lbls (int32) from HBM (probs on the sync HWDGE ring,
lbls on the scalar ring). Memory-bound: every compute engine is kept well
under the DMA budget -- including the PE at its slow 1.2 GHz p-state, so a
clock-gated PE can never backpressure the input stream.

Work split per [128, F] tile (engine busy vs 5.5us DMA budget at F=2048):
  ACT  (~2.6us): lb = bf16(l) with fused accum -> Npos = sum(l)
                 pb = bf16(p) into comb[:, :F]
  DVE  (~3.2us): y-stt:  (p > .5) * p -> comb[:, F:], accum -> Y
                 lg-stt: (p > .5) * lb -> junk,       accum -> C
  PE   (~2.0us at 2.4GHz, ~3.6us at 1.2GHz): diag trick over the 2-plane
    comb: lhsT = lb chunk [128,128], rhs = [pb_c | y_c] [128, 2*128],
    accumulated into PSUM (two banks, alternating chunks); diag of block 0
    gives Sx = sum(l*pb), block 1 gives TP = sum(l*y).
A short burst of zero matmuls warms the PE clock during the first DMAs.
First/last tiles are tapered to shorten pipeline fill and drain.

Host combine (float64):
  FP = Y - TP;  FN = Npos - Sx - C + TP
  f1 from TP/FP/FN with eps=1e-5;  return -f1 as float32 scalar.
"""

from contextlib import ExitStack

import numpy as np

N_TOTAL = 16777216
N_CORES = 8
M_PER_CORE = N_TOTAL // N_CORES   # 2097152
P = 128                           # SBUF partitions
EPS = 1e-05
CH = 128                          # diag chunk columns

_CACHE = {}


def build_nc(M=M_PER_CORE, F=2048, bufs=4, in_bufs=6, warmup_mms=12,
             debug=False):
    import concourse.bacc as bacc
    import concourse.mybir as mybir
    import concourse.tile as tile

    cols = M // P                 # 16384
    start_taper = [F // 4, F // 4]
    end_taper = [F // 4, F // 8, F // 8]
    body = (cols - sum(start_taper) - sum(end_taper)) // F
    tiles = start_taper + [F] * body + end_taper
    assert sum(tiles) == cols and all(Ft % CH == 0 for Ft in tiles)
    T = len(tiles)

    f32 = mybir.dt.float32
    i32 = mybir.dt.int32
    bf16 = mybir.dt.bfloat16
    Alu = mybir.AluOpType
    Act = mybir.ActivationFunctionType

    nc = bacc.Bacc("TRN2", target_bir_lowering=False, debug=debug,
                   num_devices=N_CORES)

    probs = nc.dram_tensor("probs", [M], f32, kind="ExternalInput")
    lbls = nc.dram_tensor("lbls", [M], i32, kind="ExternalInput")
    out_diag = nc.dram_tensor("out_diag", [P, 2 * 2 * CH], bf16,
                              kind="ExternalOutput")
    out_accN = nc.dram_tensor("out_accN", [P, T], f32, kind="ExternalOutput")
    out_accV = nc.dram_tensor("out_accV", [P, 2 * T], f32,
                              kind="ExternalOutput")

    # per-tile DRAM views: tile t is one contiguous block of P*Ft elements
    def tile_view(ap_flat, start_el, Ft):
        return ap_flat[start_el:start_el + P * Ft].rearrange(
            "(p f) -> p f", p=P, f=Ft)

    p1 = probs.ap()
    l1 = lbls.ap()

    with tile.TileContext(nc) as tc, ExitStack() as ctx:
        pin = ctx.enter_context(tc.tile_pool(name="pin", bufs=in_bufs))
        lin = ctx.enter_context(tc.tile_pool(name="lin", bufs=in_bufs))
        lbpool = ctx.enter_context(tc.tile_pool(name="lbpool", bufs=3))
        cpool = ctx.enter_context(tc.tile_pool(name="cpool", bufs=bufs))
        accp = ctx.enter_context(tc.tile_pool(name="accp", bufs=1))
        psump = ctx.enter_context(tc.tile_pool(name="psump", bufs=1,
                                               space="PSUM"))

        accN = accp.tile([P, T], f32)        # Npos partials (ACT)
        accV = accp.tile([P, 2 * T], f32)    # Y | C partials (DVE)
        junk = accp.tile([P, F], bf16)       # dead lg plane output

        # two alternating diag accumulators (even / odd chunks) so
        # back-to-back accumulating matmuls don't chain on one PSUM bank
        ps_diag0 = psump.tile([P, 2 * CH], f32)
        ps_diag1 = psump.tile([P, 2 * CH], f32)

        # Warm the PE HAM clock-gate while the first input DMAs stream.
        if warmup_mms:
            wu = accp.tile([P, 2 * CH], bf16)
            nc.vector.memset(wu[:], 0.0)
            ps_wu = psump.tile([P, 2 * CH], f32)
            for i in range(warmup_mms):
                nc.tensor.matmul(ps_wu[:, :], wu[:, :CH], wu[:],
                                 start=(i == 0), stop=(i == warmup_mms - 1))

        nctot = cols // CH              # total diag chunks (128)
        bank = [i % 2 for i in range(nctot)]
        b0_stop = max(i for i, b in enumerate(bank) if b == 0)
        b1_stop = max(i for i, b in enumerate(bank) if b == 1)
        ci = 0
        off = 0
        diag_sb = accp.tile([P, 2 * 2 * CH], bf16)
        for t, Ft in enumerate(tiles):
            NCt = Ft // CH
            start_el = P * off
            off += Ft

            # Taper tiles get their own slot sets so their DMAs queue
            # immediately at the start and prefetch early at the end.
            tap = "_tap" if Ft < F else ""
            pt = pin.tile([P, Ft], f32, tag="pt" + tap, name=f"pt{t}")
            nc.sync.dma_start(out=pt[:, :Ft], in_=tile_view(p1, start_el, Ft))
            lt = lin.tile([P, Ft], i32, tag="lt" + tap, name=f"lt{t}")
            nc.scalar.dma_start(out=lt[:, :Ft], in_=tile_view(l1, start_el, Ft))

            # ACT: lb = bf16(l) with fused accum -> Npos; pb into comb
            lb = lbpool.tile([P, F], bf16, tag="lb")
            nc.scalar.activation(lb[:, :Ft], lt[:, :Ft], Act.Copy,
                                 accum_out=accN[:, t:t + 1])
            comb = cpool.tile([P, 2 * F], bf16, tag="comb")
            nc.scalar.activation(comb[:, :Ft], pt[:, :Ft], Act.Copy)

            # DVE: y plane + Y accum; lg (dead) + C accum
            nc.vector.scalar_tensor_tensor(
                out=comb[:, F:F + Ft], in0=pt[:, :Ft], scalar=0.5,
                in1=pt[:, :Ft], op0=Alu.is_gt, op1=Alu.mult,
                accum_out=accV[:, t:t + 1])
            nc.vector.scalar_tensor_tensor(
                out=junk[:, :Ft], in0=pt[:, :Ft], scalar=0.5,
                in1=lb[:, :Ft], op0=Alu.is_gt, op1=Alu.mult,
                accum_out=accV[:, T + t:T + t + 1])

            # PE diag: ps += lb_c.T @ [pb_c | y_c]
            comb_r = comb[:].rearrange("p (s x) -> p s x", s=2, x=F)
            for c in range(NCt):
                ps = ps_diag0 if bank[ci] == 0 else ps_diag1
                nc.tensor.matmul(
                    ps[:, :], lb[:, c * CH:(c + 1) * CH],
                    comb_r[:, :, c * CH:(c + 1) * CH],
                    start=(ci in (0, 1)),
                    stop=(ci in (b0_stop, b1_stop)))
                ci += 1

        # PSUM -> SBUF (bf16 halves the output DMA) -> DRAM
        nc.scalar.activation(diag_sb[:, :2 * CH], ps_diag0[:, :], Act.Copy)
        nc.vector.tensor_copy(out=diag_sb[:, 2 * CH:], in_=ps_diag1[:, :])

        nc.sync.dma_start(out=out_diag.ap(), in_=diag_sb[:])
        nc.sync.dma_start(out=out_accV.ap(), in_=accV[:])
        nc.scalar.dma_start(out=out_accN.ap(), in_=accN[:])

    nc.compile()
    return nc, T


def get_nc():
    if "nc" not in _CACHE:
        _CACHE["nc"] = build_nc()
    return _CACHE["nc"]


def run_device(probs, lbls, trace=False, **run_kwargs):
    """Run the SPMD kernel; returns (per-core result dicts, BassKernelResults)."""
    from concourse import bass_utils

    nc, _ = get_nc()
    probs = np.ascontiguousarray(probs, dtype=np.float32)
    lbls = np.ascontiguousarray(lbls, dtype=np.int32)
    assert probs.shape == (N_TOTAL,) and lbls.shape == (N_TOTAL,)
    M = M_PER_CORE
    in_maps = [
        {"probs": probs[c * M:(c + 1) * M], "lbls": lbls[c * M:(c + 1) * M]}
        for c in range(N_CORES)
    ]
    res = bass_utils.run_bass_kernel_spmd(
        nc, in_maps, core_ids=list(range(N_CORES)), trace=trace, **run_kwargs)
    return res.results, res


def combine(results):
    """Combine per-core partial sums into the final -f1 scalar."""
    _, T = get_nc()
    Npos = Y = C = Sx = TP = 0.0
    for r in results:
        dg = np.asarray(r["out_diag"], dtype=np.float64).reshape(P, 2, 2, CH)
        for b in range(2):
            Sx += np.trace(dg[:, b, 0, :])
            TP += np.trace(dg[:, b, 1, :])
        Npos += np.asarray(r["out_accN"], dtype=np.float64).sum()
        v = np.asarray(r["out_accV"], dtype=np.float64)
        Y += v[:, :T].sum()
        C += v[:, T:].sum()

    FP = Y - TP
    FN = Npos - Sx - C + TP
    precision = (TP + EPS) / (TP + FP + EPS)
    recall = (TP + EPS) / (TP + FN + EPS)
    f1 = 2.0 * precision * recall / (precision + recall)
    return np.float32(-f1)


def kernel(probs, lbls):
    results, _ = run_device(probs, lbls)
    return np.asarray(combine(results), dtype=np.float32)


if __name__ == "__main__":
    rng = np.random.default_rng(0)
    p = rng.uniform(0, 1, N_TOTAL).astype(np.float32)
    l = rng.integers(0, 2, N_TOTAL).astype(np.int32)
    out = kernel(p, l)
    print("kernel output:", out)
